# revision 2
# baseline (speedup 1.0000x reference)
"""Trainium2 Bass kernel for nn_Encoder_LaplaceGNN_PPISAGE (3-layer GraphSAGE
encoder with graph-mode LayerNorm + PReLU + skip connections).

Strategy (8 NeuronCores, SPMD):
- Nodes partitioned contiguously: core c owns rows [c*12500, (c+1)*12500).
- Mean aggregation per layer = one-hot matmul: gathered edge messages
  [128 edges, 128 d] (bf16, dma_gather from a replicated node-feature table)
  are lhsT; selection matrix S [128 edges, 128 window-nodes] (bf16, built
  on DVE via dual-op tensor_scalar: is_equal(iota, dstcol) * inv_cnt) is rhs;
  accumulated over the window's edge groups into PSUM meanT [d, 128 nodes].
- dma_gather int16 indices: the gather ucode TRIMS trailing negative
  indices, and a call whose indices are all negative wedges the device.
  So the 100000-row table is split into FOUR 25000-row chunks; per-chunk
  relative indices are always in [0, 25000) -- no negative index ever.
  Pad slots use idx 0 (valid row).
- Gather calls are capped by the SWDGE descriptor ring (1024 descs): at
  most 8 128-slot groups per call. Calls are cycled over SWDGE queues 1-3
  so descriptor generation (994ns fixed + 0.34ns/desc on Q7) runs in
  parallel across queues and stays hidden under the DMA drain.
- h = meanT.T @ Wl + rootT.T @ Wr in PSUM (node-major out). Bias is folded
  into the LayerNorm affine (stats corrected analytically).
- Graph LayerNorm stats: per-core [chsum(128), sum, sumsq] -> AllReduce.
- z tables for the next layer's gather: bf16, AllGather'd across cores.
  The skip connections use the ORIGINAL x (kept resident as xT) per the
  reference: z1 = h1 + x@Ws1, z2 = h1 + h2 + x@Ws2.
"""
import os
import sys

_TRN_REPO = "/opt/trn_rl_repo"
if _TRN_REPO not in sys.path:
    sys.path.insert(0, _TRN_REPO)

import numpy as np
import ml_dtypes

N = 100000
E = 1600000
D_IN = 50
D = 128
EPS = 1e-5
NCORES = 8
NLOC = N // NCORES          # 12500
W = (NLOC + 127) // 128     # 98 windows/core
NPAD = W * 128              # 12544
CW = 25000                  # chunk width; 4 chunks, offsets in [0, CW) >= 0
NCH = 4
BW = 4                      # windows per batch
MAXG_CALL = 8               # SWDGE ring holds 1024 descs -> max 8 groups per gather call


def _bf16(a):
    return np.asarray(a).astype(ml_dtypes.bfloat16)


def _build_schedule(edge_src, edge_dst):
    """Host-side edge schedule with variable per-(window, chunk) group counts.

    Slot order per batch of BW windows: [C0(w0..) | C1(..) | C2 | C3],
    each (window, chunk) run padded to a multiple of 128 slots. All
    relative indices are nonnegative (4 chunks of 25000 rows), so the
    gather ucode's trailing-negative trim can never fire.
    """
    src = np.asarray(edge_src).astype(np.int64)
    dst = np.asarray(edge_dst).astype(np.int64)
    core = dst // NLOC
    loc = dst % NLOC
    win = loc // 128
    col = loc % 128
    chunk = src // CW
    iv_all = src - chunk * CW

    cnt = np.bincount(dst, minlength=N).astype(np.float32)
    inv_cnt = (1.0 / np.maximum(cnt, 1.0)).astype(np.float32)

    # group counts per (core, window, chunk)
    key = (core * W + win) * NCH + chunk
    kcnt = np.bincount(key, minlength=NCORES * W * NCH).reshape(NCORES, W, NCH)
    g = np.ceil(kcnt / 128).astype(np.int64)          # [C, W, NCH]
    # SPMD: one NEFF for all cores -> use the max group count per (w, ch)
    g_max = g.max(axis=0)                              # [W, NCH]

    nbatch = (W + BW - 1) // BW
    batches = []
    group_base = 0
    slot_base = 0
    for b in range(nbatch):
        w0 = b * BW
        wl = list(range(w0, min(w0 + BW, W)))
        gcs = [[int(g_max[w, c]) for w in wl] for c in range(NCH)]  # [NCH][BW]
        spans = [sum(gc) for gc in gcs]
        batches.append(dict(windows=wl, gcs=gcs, spans=spans,
                            group_base=group_base, slot_base=slot_base))
        group_base += sum(spans)
        slot_base += sum(spans) * 128
    NG = group_base
    SLOTS = slot_base

    # slot assignment: order edges by (core, window, chunk); stable within
    order = np.lexsort((chunk, win, core))
    s_win = win[order]
    s_col = col[order]
    s_chunk = chunk[order]
    s_core = core[order]
    s_dst = dst[order]
    s_iv = iv_all[order]

    # within-run position
    runs = kcnt.reshape(-1)
    within = np.arange(E, dtype=np.int64) - np.repeat(
        np.concatenate([[0], np.cumsum(runs)[:-1]]), runs)

    # run start slot (shared layout across cores)
    run_slot0 = np.zeros((W, NCH), np.int64)
    for binfo in batches:
        off = binfo["slot_base"]
        for c in range(NCH):
            for j, w in enumerate(binfo["windows"]):
                run_slot0[w, c] = off + sum(binfo["gcs"][c][:j]) * 128
            off += binfo["spans"][c] * 128

    slot = run_slot0[s_win, s_chunk] + within

    idxval = np.zeros((NCORES, SLOTS), np.int16)          # pad -> 0 (valid row)
    dstcol = np.full((NCORES, SLOTS), -1.0, np.float32)   # pad -> -1 (no column)
    invw = np.zeros((NCORES, SLOTS), np.float32)
    idxval[s_core, slot] = s_iv.astype(np.int16)
    dstcol[s_core, slot] = s_col.astype(np.float32)
    invw[s_core, slot] = inv_cnt[s_dst]
    assert (idxval >= 0).all()

    # wrapped int16 layout: slot s -> [s%16 (replicated x8), s//16]
    F = SLOTS // 16
    idx16 = np.ascontiguousarray(
        idxval.reshape(NCORES, F, 16).transpose(0, 2, 1))      # [C,16,F]
    idx16 = np.tile(idx16, (1, 8, 1))                          # [C,128,F]
    # per-group strips: slot s -> [s%128, s//128]
    dstcol_s = np.ascontiguousarray(
        dstcol.reshape(NCORES, NG, 128).transpose(0, 2, 1))    # [C,128,NG]
    invw_s = np.ascontiguousarray(
        invw.reshape(NCORES, NG, 128).transpose(0, 2, 1))
    gbmax = max(sum(b["spans"]) for b in batches)
    return dict(batches=batches, NG=NG, SLOTS=SLOTS, gbmax=gbmax,
                idx16=idx16, dstcol=dstcol_s, invw=invw_s)


def _build_nc(sched_key, batches, NG, gbmax, alphas, Sb, Sbb, has_bias):
    NL = int(os.environ.get("K_NLAYERS", "3"))
    import concourse.bacc as bacc
    import concourse.tile as tile
    import concourse.mybir as mybir

    F32 = mybir.dt.float32
    BF16 = mybir.dt.bfloat16
    I16 = mybir.dt.int16
    AF = mybir.ActivationFunctionType
    OP = mybir.AluOpType

    FTOT = NG * 8

    nc = bacc.Bacc("TRN2", target_bir_lowering=False, debug=False,
                   num_devices=NCORES, enable_partition_id=False,
                   num_swdge_queues=4)

    x_tab = nc.dram_tensor("x_tab", [N, D], BF16, kind="ExternalInput")
    xT_in = nc.dram_tensor("xT_in", [128, NPAD], BF16, kind="ExternalInput")
    idx_in = nc.dram_tensor("idx_in", [128, FTOT], I16, kind="ExternalInput")
    dst_in = nc.dram_tensor("dst_in", [128, NG], F32, kind="ExternalInput")
    inv_in = nc.dram_tensor("inv_in", [128, NG], F32, kind="ExternalInput")
    iota_in = nc.dram_tensor("iota_in", [128, 128], BF16, kind="ExternalInput")
    ident_in = nc.dram_tensor("ident_in", [128, 128], BF16, kind="ExternalInput")
    Wl_in = [nc.dram_tensor(f"Wl{i}", [128, 128], BF16, kind="ExternalInput") for i in range(3)]
    Wr_in = [nc.dram_tensor(f"Wr{i}", [128, 128], BF16, kind="ExternalInput") for i in range(3)]
    Ws_in = [nc.dram_tensor(f"Ws{i}", [128, 128], BF16, kind="ExternalInput") for i in range(2)]
    bcol_in = [nc.dram_tensor(f"bcol{i}", [128, 1], F32, kind="ExternalInput") for i in range(3)]
    brow_in = [nc.dram_tensor(f"brow{i}", [1, 128], F32, kind="ExternalInput") for i in range(3)]
    lnw_in = [nc.dram_tensor(f"lnw{i}", [1, 128], F32, kind="ExternalInput") for i in range(3)]
    lnb_in = [nc.dram_tensor(f"lnb{i}", [1, 128], F32, kind="ExternalInput") for i in range(3)]
    ones_in = nc.dram_tensor("ones_in", [128, 1], BF16, kind="ExternalInput")

    ret_out = nc.dram_tensor("ret_out", [NLOC, D], F32, kind="ExternalOutput")

    # internal DRAM
    zshard = [nc.dram_tensor(f"zshard{i}", [NLOC, D], BF16, kind="Internal")
              for i in range(2)]
    # ztab NOT addr_space="Shared": dma_gather reads it, and gathering from
    # the Shared scratchpad window wedges the device.
    ztab = [nc.dram_tensor(f"ztab{i}", [N, D], BF16, kind="Internal")
            for i in range(2)]
    st_in = nc.dram_tensor("st_in", [130, 1], F32, kind="Internal")
    st_out = [nc.dram_tensor(f"st_out{i}", [130, 1], F32, kind="Internal",
                             addr_space="Shared") for i in range(3)]
    rowbounce = nc.dram_tensor("rowbounce", [1, 256], F32, kind="Internal")

    with tile.TileContext(nc) as tc:
        import contextlib
        with contextlib.ExitStack() as ctx:
            # persistent pools
            pers = ctx.enter_context(tc.tile_pool(name="pers", bufs=1))
            xT = pers.tile([128, NPAD], BF16)        # original x^T (skip matmuls)
            rootT = pers.tile([128, NPAD], BF16)     # layer-l root features, T-layout
            hcur = pers.tile([128, NPAD], BF16)      # current layer pre-LN h (node-major)
            h1s = pers.tile([128, NPAD], BF16)       # post-prelu h1 (node-major)
            iota_t = pers.tile([128, 128], BF16)
            ident_t = pers.tile([128, 128], BF16)
            dst_t = pers.tile([128, NG], F32)
            inv_t = pers.tile([128, NG], F32)
            ones_t = pers.tile([128, 1], BF16)
            Wl_t = [pers.tile([128, 128], BF16, tag=f"wl{i}", name=f"wl{i}") for i in range(3)]
            Wr_t = [pers.tile([128, 128], BF16, tag=f"wr{i}", name=f"wr{i}") for i in range(3)]
            Ws_t = [pers.tile([128, 128], BF16, tag=f"ws{i}", name=f"ws{i}") for i in range(2)]
            bcol_t = [pers.tile([128, 1], F32, tag=f"bc{i}", name=f"bc{i}") for i in range(3)]
            brow_t = [pers.tile([1, 128], F32, tag=f"br{i}", name=f"br{i}") for i in range(3)]
            lnw_t = [pers.tile([1, 128], F32, tag=f"lw{i}", name=f"lw{i}") for i in range(3)]
            lnb_t = [pers.tile([1, 128], F32, tag=f"lb{i}", name=f"lb{i}") for i in range(3)]

            nc.sync.dma_start(xT[:], xT_in[:])
            nc.vector.tensor_copy(rootT[:], xT[:])
            nc.sync.dma_start(iota_t[:], iota_in[:])
            nc.sync.dma_start(ident_t[:], ident_in[:])
            nc.sync.dma_start(dst_t[:], dst_in[:])
            nc.sync.dma_start(inv_t[:], inv_in[:])
            nc.sync.dma_start(ones_t[:], ones_in[:])
            for i in range(3):
                nc.sync.dma_start(Wl_t[i][:], Wl_in[i][:])
                nc.sync.dma_start(Wr_t[i][:], Wr_in[i][:])
                nc.sync.dma_start(bcol_t[i][:], bcol_in[i][:])
                nc.sync.dma_start(brow_t[i][:], brow_in[i][:])
                nc.sync.dma_start(lnw_t[i][:], lnw_in[i][:])
                nc.sync.dma_start(lnb_t[i][:], lnb_in[i][:])
            for i in range(2):
                nc.sync.dma_start(Ws_t[i][:], Ws_in[i][:])

            self_qcnt = [0]
            for layer in range(NL):
                tab = x_tab if layer == 0 else ztab[layer - 1]
                tabs = [tab[c * CW:, :] for c in range(NCH)]

                # ---------------- pass 1: aggregate + h ----------------
                with tc.tile_pool(name=f"p1s_{layer}", bufs=2) as wp, \
                     tc.tile_pool(name=f"p1S_{layer}", bufs=4) as sp, \
                     tc.tile_pool(name=f"p1m_{layer}", bufs=2, space="PSUM") as mps, \
                     tc.tile_pool(name=f"p1h_{layer}", bufs=2, space="PSUM") as hps, \
                     tc.tile_pool(name=f"p1c_{layer}", bufs=1, space="PSUM") as cps:
                    sumS = wp.tile([128, W], F32, tag="sums", bufs=1)
                    sqS = wp.tile([128, W], F32, tag="sqs", bufs=1)
                    sqscr = wp.tile([128, 128], F32, tag="sqscr", bufs=2)
                    if has_bias:
                        chcol = wp.tile([128, W], F32, tag="chcol", bufs=1)

                    maxb = int(os.environ.get("K_MAXB", "0")) or len(batches)
                    for binfo in batches[:maxb]:
                        g0 = binfo["group_base"]
                        spans = binfo["spans"]
                        ngr = sum(spans)
                        f0 = g0 * 8
                        idx_t = wp.tile([128, ngr * 8], I16, tag="idx")
                        nc.sync.dma_start(idx_t[:], idx_in[:, f0:f0 + ngr * 8])
                        msg = wp.tile([128, gbmax, 128], BF16, tag="msg")
                        if os.environ.get("K_SKIP_GATHER"):
                            nc.vector.memset(msg[:], 0.0)
                            gather_spans = ()
                        else:
                            offs = np.concatenate([[0], np.cumsum(spans)]).astype(int)
                            gather_spans = tuple(
                                (tabs[c], int(offs[c]), int(offs[c + 1]))
                                for c in range(NCH))
                        for tab_ap, lo, hi in gather_spans:
                            span = hi - lo
                            if not span:
                                continue
                            ncall = (span + MAXG_CALL - 1) // MAXG_CALL
                            szs = [span // ncall + (1 if i < span % ncall else 0)
                                   for i in range(ncall)]
                            o = lo
                            for sz in szs:
                                nc.gpsimd.dma_gather(
                                    msg[:, o:o + sz, :], tab_ap,
                                    idx_t[:, o * 8:(o + sz) * 8],
                                    sz * 128, sz * 128, D,
                                    queue_num=(1 + self_qcnt[0] % 3) if not os.environ.get("K_ONEQ") else 0)
                                self_qcnt[0] += 1
                                o += sz
                        for j, w in enumerate(binfo["windows"]):
                            # local group indices for this window across chunks
                            lgs = []
                            off = 0
                            for c in range(NCH):
                                lo_c = off + sum(binfo["gcs"][c][:j])
                                lgs.extend(range(lo_c, lo_c + binfo["gcs"][c][j]))
                                off += spans[c]
                            mean_ps = mps.tile([128, 128], F32, space="PSUM",
                                               tag="mps", padded_shape=[128, 512])
                            for k, lg in enumerate(lgs):
                                gg = g0 + lg
                                s_t = sp.tile([128, 128], BF16, tag="s")
                                nc.vector.tensor_scalar(
                                    out=s_t[:], in0=iota_t[:],
                                    scalar1=dst_t[:, gg:gg + 1],
                                    scalar2=inv_t[:, gg:gg + 1],
                                    op0=OP.is_equal, op1=OP.mult)
                                lhs_g = s_t if os.environ.get("K_NOMSG") else msg[:, lg, :]
                                nc.tensor.matmul(mean_ps[:], lhsT=lhs_g,
                                                 rhs=s_t[:], start=(k == 0),
                                                 stop=(k == len(lgs) - 1))
                            meanT = wp.tile([128, 128], BF16, tag="meanT")
                            nc.scalar.copy(meanT[:], mean_ps[:])
                            h_ps = hps.tile([128, 128], F32, space="PSUM",
                                            tag="hps", padded_shape=[128, 512])
                            ws = w * 128
                            nc.tensor.matmul(h_ps[:], lhsT=meanT[:], rhs=Wl_t[layer][:],
                                             start=True, stop=False)
                            nc.tensor.matmul(h_ps[:], lhsT=rootT[:, ws:ws + 128],
                                             rhs=Wr_t[layer][:], start=False, stop=True)
                            nc.scalar.activation(hcur[:, ws:ws + 128], h_ps[:],
                                                 AF.Copy, accum_out=sumS[:, w:w + 1])
                            nc.scalar.activation(sqscr[:], hcur[:, ws:ws + 128],
                                                 AF.Square, accum_out=sqS[:, w:w + 1])
                            if has_bias:
                                ch_ps = cps.tile([128, 1], F32, space="PSUM",
                                                 tag="chps", padded_shape=[128, 512])
                                nc.tensor.matmul(ch_ps[:], lhsT=hcur[:, ws:ws + 128],
                                                 rhs=ones_t[:], start=True, stop=True)
                                nc.scalar.copy(chcol[:, w:w + 1], ch_ps[:])

                    # stats -> [130,1] DRAM, AllReduce
                    red = wp.tile([128, 2], F32, tag="red", bufs=1)
                    nc.vector.reduce_sum(red[:, 0:1], sumS[:], axis=mybir.AxisListType.X)
                    nc.vector.reduce_sum(red[:, 1:2], sqS[:], axis=mybir.AxisListType.X)
                    stat2 = cps.tile([2, 1], F32, space="PSUM", tag="st2",
                                     padded_shape=[2, 512])
                    ones_f = wp.tile([128, 1], F32, tag="onesf", bufs=1)
                    nc.vector.memset(ones_f[:], 1.0)
                    nc.tensor.matmul(stat2[:], lhsT=red[:], rhs=ones_f[:],
                                     start=True, stop=True)
                    s2_sb = wp.tile([2, 1], F32, tag="s2sb", bufs=1)
                    nc.scalar.copy(s2_sb[:], stat2[:])
                    nc.sync.dma_start(st_in[128:130, :], s2_sb[:])
                    if has_bias:
                        ch_sb = wp.tile([128, 1], F32, tag="chsb", bufs=1)
                        nc.vector.reduce_sum(ch_sb[:], chcol[:], axis=mybir.AxisListType.X)
                        nc.sync.dma_start(st_in[0:128, :], ch_sb[:])

                nc.gpsimd.collective_compute(
                    "AllReduce", OP.add, replica_groups=[list(range(NCORES))],
                    ins=[st_in[:]], outs=[st_out[layer][:]])

                # ---------------- LN scalars ----------------
                with tc.tile_pool(name=f"ln_{layer}", bufs=1) as lp, \
                     tc.tile_pool(name=f"lnp_{layer}", bufs=1, space="PSUM") as lps:
                    ar_s = lp.tile([1, 1], F32)
                    ar_sq = lp.tile([1, 1], F32)
                    nc.sync.dma_start(ar_s[:], st_out[layer][128:129, :])
                    nc.sync.dma_start(ar_sq[:], st_out[layer][129:130, :])
                    ND = float(N * D)
                    sc = lp.tile([1, 8], F32, tag="sc")
                    # sc0 = mu' = sum/ND + N*Sb/ND
                    nc.vector.tensor_scalar(
                        out=sc[:, 0:1], in0=ar_s[:], scalar1=1.0 / ND,
                        scalar2=float(N) * Sb[layer] / ND, op0=OP.mult, op1=OP.add)
                    # sc1 = sumsq/ND + N*Sbb/ND
                    nc.vector.tensor_scalar(
                        out=sc[:, 1:2], in0=ar_sq[:], scalar1=1.0 / ND,
                        scalar2=float(N) * Sbb[layer] / ND, op0=OP.mult, op1=OP.add)
                    if has_bias:
                        ar_ch = lp.tile([128, 1], F32)
                        nc.sync.dma_start(ar_ch[:], st_out[layer][0:128, :])
                        ar_ch_bf = lp.tile([128, 1], BF16, tag="archbf")
                        bcol_bf = lp.tile([128, 1], BF16, tag="bcolbf")
                        nc.vector.tensor_copy(ar_ch_bf[:], ar_ch[:])
                        nc.vector.tensor_copy(bcol_bf[:], bcol_t[layer][:])
                        dot_ps = lps.tile([1, 1], F32, space="PSUM", padded_shape=[1, 512])
                        nc.tensor.matmul(dot_ps[:], lhsT=ar_ch_bf[:],
                                         rhs=bcol_bf[:], start=True, stop=True)
                        # sc2 = sc1 + dot*2/ND  (E[(h+b)^2])
                        nc.vector.tensor_scalar(
                            out=sc[:, 2:3], in0=dot_ps[:], scalar1=2.0 / ND,
                            scalar2=None, op0=OP.mult)
                        nc.vector.tensor_tensor(out=sc[:, 2:3], in0=sc[:, 2:3],
                                                in1=sc[:, 1:2], op=OP.add)
                    else:
                        nc.vector.tensor_copy(sc[:, 2:3], sc[:, 1:2])
                    # sc3 = mu'^2 ; sc4 = var = sc2 - sc3
                    nc.scalar.square(sc[:, 3:4], sc[:, 0:1])
                    nc.vector.tensor_tensor(out=sc[:, 4:5], in0=sc[:, 2:3],
                                            in1=sc[:, 3:4], op=OP.subtract)
                    # sc5 = sqrt(var) + EPS ; sc6 = 1/sc5
                    nc.scalar.sqrt(sc[:, 5:6], sc[:, 4:5])
                    nc.vector.tensor_scalar(out=sc[:, 5:6], in0=sc[:, 5:6],
                                            scalar1=EPS, scalar2=None, op0=OP.add)
                    nc.vector.reciprocal(sc[:, 6:7], sc[:, 5:6])
                    # scaleRow = lnw * inv_std ; biasRow = (b - mu')*scaleRow + lnb
                    srow = lp.tile([1, 128], F32, tag="srow")
                    brow2 = lp.tile([1, 128], F32, tag="brow2")
                    nc.vector.tensor_scalar(out=srow[:], in0=lnw_t[layer][:],
                                            scalar1=sc[:, 6:7], scalar2=None,
                                            op0=OP.mult)
                    nc.vector.tensor_scalar(out=brow2[:], in0=brow_t[layer][:],
                                            scalar1=sc[:, 0:1], scalar2=None,
                                            op0=OP.subtract)
                    nc.vector.tensor_tensor(out=brow2[:], in0=brow2[:], in1=srow[:],
                                            op=OP.mult)
                    nc.vector.tensor_tensor(out=brow2[:], in0=brow2[:],
                                            in1=lnb_t[layer][:], op=OP.add)
                    # broadcast rows to [128,128] via DRAM bounce
                    nc.sync.dma_start(rowbounce[:, 0:128], srow[:])
                    nc.sync.dma_start(rowbounce[:, 128:256], brow2[:])
                    scale_bc = lp.tile([128, 128], F32, tag="scbc")
                    bias_bc = lp.tile([128, 128], F32, tag="bibc")
                    nc.gpsimd.dma_start(
                        out=scale_bc[:], in_=rowbounce[:, 0:128].to_broadcast([128, 128]))
                    nc.gpsimd.dma_start(
                        out=bias_bc[:], in_=rowbounce[:, 128:256].to_broadcast([128, 128]))

                    # ---------------- pass 2: LN + PReLU + z/ret ----------------
                    with tc.tile_pool(name=f"p2_{layer}", bufs=3) as p2, \
                         tc.tile_pool(name=f"p2p_{layer}", bufs=2, space="PSUM") as zps, \
                         tc.tile_pool(name=f"p2t_{layer}", bufs=2, space="PSUM") as tps:
                        for w in range(W):
                            ws = w * 128
                            nrow = 128 if w < W - 1 else NLOC - ws
                            y = p2.tile([128, 128], F32, tag="y")
                            nc.vector.tensor_tensor(out=y[:], in0=hcur[:, ws:ws + 128],
                                                    in1=scale_bc[:], op=OP.mult)
                            nc.vector.tensor_tensor(out=y[:], in0=y[:],
                                                    in1=bias_bc[:], op=OP.add)
                            post = p2.tile([128, 128], F32, tag="post")
                            nc.scalar.activation(post[:], y[:], AF.Prelu,
                                                 alpha=alphas[layer])
                            if layer == NL - 1:
                                nc.sync.dma_start(ret_out[ws:ws + nrow, :],
                                                  post[:nrow, :])
                                continue
                            # z build (skip connections use the ORIGINAL x)
                            z_ps = zps.tile([128, 128], F32, space="PSUM", tag="z",
                                            padded_shape=[128, 512])
                            nc.tensor.matmul(z_ps[:], lhsT=xT[:, ws:ws + 128],
                                             rhs=Ws_t[layer][:], start=True, stop=True)
                            z_sb = p2.tile([128, 128], BF16, tag="zsb")
                            nc.vector.tensor_tensor(out=z_sb[:], in0=z_ps[:],
                                                    in1=post[:], op=OP.add)
                            if layer == 0:
                                nc.vector.tensor_copy(h1s[:, ws:ws + 128], post[:])
                            else:
                                nc.vector.tensor_tensor(out=z_sb[:], in0=z_sb[:],
                                                        in1=h1s[:, ws:ws + 128],
                                                        op=OP.add)
                            nc.sync.dma_start(zshard[layer][ws:ws + nrow, :],
                                              z_sb[:nrow, :])
                            # transpose into rootT for next layer
                            t_ps = tps.tile([128, 128], BF16, space="PSUM", tag="t",
                                            padded_shape=[128, 1024])
                            nc.tensor.transpose(t_ps[:], z_sb[:], ident_t[:])
                            nc.scalar.copy(rootT[:, ws:ws + 128], t_ps[:])
                        if layer < NL - 1:
                            nc.vector.memset(rootT[:, NLOC:NPAD], 0.0)

                if layer < NL - 1:
                    nc.gpsimd.collective_compute(
                        "AllGather", mybir.AluOpType.bypass,
                        replica_groups=[list(range(NCORES))],
                        ins=[zshard[layer][:]], outs=[ztab[layer][:]])

    nc.compile()
    return nc


def _prep_inputs(inputs, sched):
    """Build per-core in_maps."""
    x = np.asarray(inputs["x"], np.float32)
    x_tab = np.zeros((N, D), ml_dtypes.bfloat16)
    x_tab[:, :D_IN] = _bf16(x)

    def padW(a):  # [din, dout] -> [128,128] zero-padded
        out = np.zeros((128, 128), np.float32)
        out[:a.shape[0], :a.shape[1]] = np.asarray(a, np.float32)
        return out

    Wl = [padW(inputs["Wl1"]), padW(inputs["Wl2"]), padW(inputs["Wl3"])]
    Wr = [padW(inputs["Wr1"]), padW(inputs["Wr2"]), padW(inputs["Wr3"])]
    Ws = [padW(inputs["Ws1"]), padW(inputs["Ws2"])]
    b = [np.asarray(inputs[k], np.float32) for k in ("b1", "b2", "b3")]
    lnw = [np.asarray(inputs[k], np.float32) for k in ("lnw1", "lnw2", "lnw3")]
    lnb = [np.asarray(inputs[k], np.float32) for k in ("lnb1", "lnb2", "lnb3")]

    iota = np.tile(np.arange(128, dtype=ml_dtypes.bfloat16)[None, :], (128, 1))
    ident = np.eye(128, dtype=ml_dtypes.bfloat16)
    ones_col = np.ones((128, 1), ml_dtypes.bfloat16)

    common = dict(x_tab=x_tab, iota_in=iota, ident_in=ident, ones_in=ones_col)
    for i in range(3):
        common[f"Wl{i}"] = _bf16(Wl[i])
        common[f"Wr{i}"] = _bf16(Wr[i])
        common[f"bcol{i}"] = b[i].reshape(128, 1)
        common[f"brow{i}"] = b[i].reshape(1, 128)
        common[f"lnw{i}"] = lnw[i].reshape(1, 128)
        common[f"lnb{i}"] = lnb[i].reshape(1, 128)
    for i in range(2):
        common[f"Ws{i}"] = _bf16(Ws[i])

    in_maps = []
    for c in range(NCORES):
        xT = np.zeros((128, NPAD), ml_dtypes.bfloat16)
        xT[:D_IN, :NLOC] = _bf16(x[c * NLOC:(c + 1) * NLOC, :].T)
        m = dict(common)
        m["xT_in"] = xT
        m["idx_in"] = sched["idx16"][c]
        m["dst_in"] = sched["dstcol"][c]
        m["inv_in"] = sched["invw"][c]
        in_maps.append(m)
    return in_maps


_CACHE = {}


def kernel(**inputs) -> np.ndarray:
    sched = _build_schedule(inputs["edge_src"], inputs["edge_dst"])
    alphas = [float(inputs["a1"]), float(inputs["a2"]), float(inputs["a3"])]
    b_arrs = [np.asarray(inputs[k], np.float64) for k in ("b1", "b2", "b3")]
    Sb = [float(a.sum()) for a in b_arrs]
    Sbb = [float((a * a).sum()) for a in b_arrs]
    has_bias = any(s != 0.0 for s in Sb + Sbb)

    key = (tuple(tuple(map(tuple, b["gcs"])) for b in sched["batches"]),
           tuple(alphas), tuple(Sb), tuple(Sbb), has_bias)
    if key not in _CACHE:
        _CACHE[key] = _build_nc(key, sched["batches"], sched["NG"],
                                sched["gbmax"], alphas, Sb, Sbb, has_bias)
    nc = _CACHE[key]

    in_maps = _prep_inputs(inputs, sched)
    from concourse.bass_utils import run_bass_kernel_spmd
    res = run_bass_kernel_spmd(nc, in_maps, core_ids=list(range(NCORES)))
    out = np.concatenate([r["ret_out"] for r in res.results], axis=0)
    return out.astype(np.float32)


if __name__ == "__main__":
    sys.path.insert(0, os.path.dirname(os.path.abspath(__file__)))
    import numpy as np
    dat = np.load("/tmp/ref_io.npz")
    inputs = {k: dat[k] for k in dat.files}
    got = kernel(**inputs)
    sys.stderr.write("kernel ran\n")
    # numpy reference
    x = inputs["x"].astype(np.float32)
    src, dst = inputs["edge_src"], inputs["edge_dst"]
    cnt = np.bincount(dst, minlength=N).astype(np.float32)

    def sage(h, Wl, Wr, b):
        s = np.zeros((N, h.shape[1]), np.float32)
        np.add.at(s, dst, h[src])
        mean = s / np.maximum(cnt, 1.0)[:, None]
        return mean @ Wl + h @ Wr + b

    def gln(h, w, b):
        xc = h - h.mean()
        std = np.sqrt((xc * xc).mean())
        return (xc / (std + EPS)) * w + b

    def prelu(h, a):
        return np.where(h >= 0, h, a * h)

    h1 = prelu(gln(sage(x, inputs["Wl1"], inputs["Wr1"], inputs["b1"]),
                   inputs["lnw1"], inputs["lnb1"]), inputs["a1"])
    h2 = prelu(gln(sage(h1 + x @ inputs["Ws1"], inputs["Wl2"], inputs["Wr2"], inputs["b2"]),
                   inputs["lnw2"], inputs["lnb2"]), inputs["a2"])
    NLe = int(os.environ.get("K_NLAYERS", "3"))
    if NLe == 1:
        exp = h1
    elif NLe == 2:
        exp = h2
    else:
        exp = prelu(gln(sage(h1 + h2 + x @ inputs["Ws2"], inputs["Wl3"],
                             inputs["Wr3"], inputs["b3"]),
                        inputs["lnw3"], inputs["lnb3"]), inputs["a3"])
    rel = np.linalg.norm(got - exp) / np.linalg.norm(exp)
    print("Relative error (L2):", rel)
